# revision 1
# baseline (speedup 1.0000x reference)
"""Trainium2 Bass kernel for nn_Attention_59691455480358 (sparse CLS attention).

Math: the reference computes softmax over
    logits[b, n] = (x[b,0]@W_q) . (x[b,1+n]@W_k) * C^-0.5,  n in [0, 2048).
Only the CLS query row matters and V is unused, so fold the K-projection into
the query side:

    t[b]        = W_k @ (x[b,0,:] @ W_q)          # [C] per example
    logits[b,n] = x[b,1+n,:] . t[b]               # row dot-products
    out[b]      = softmax(logits[b] * C^-0.5)

Sharding: pure data parallel — batch 16 over 8 NeuronCores (2 examples/core).
The kernel is HBM-bandwidth-bound (x is 134 MB), so x and the replicated
weights ship as bf16 (host-side cast; all on-chip accumulation is fp32 and the
output is fp32).  W_k additionally ships pre-transposed so the PE consumes
both weights directly with no on-chip weight transposes.

DMA routing: the x stream (1 MB groups) runs on the SP hardware-DGE queue;
the weights run on the separate ACT hardware-DGE queue so the t chain starts
early; small latency-critical transfers (x0^T gather, t round-trip, t
broadcast, outputs) run on the software DGE so they never queue behind
megabyte streams.

The t chain is 32 rhs-streaming bf16 matmuls (weights as the moving operand,
so no expensive 128-column LDWEIGHTS) plus 8 tiny PE transposes for q^T.
t takes a small DRAM round-trip so a stride-0 DMA can broadcast it across all
128 partitions.  The heavy pass runs on DVE via fused scalar_tensor_tensor
(out = x*t elementwise, accum_out = row sums).  Softmax: ACT exp with fused
row-sum accumulation, an all-ones PE matmul for the cross-partition sum
(every partition receives the total), DVE reciprocal + tensor_scalar multiply.

Row->partition mapping: rows 1..2048 of x[b] are viewed as [128, 16*1024]
(partition p holds rows 16p+1 .. 16p+16), so logit tile L[p, f] is the logit
for n = 16p + f and the output DMA writes 64B-contiguous runs per partition.
No max-subtraction in softmax: scaled logits are ~N(0,1) (weights are
1/sqrt(C)-scaled gaussians), exp cannot overflow fp32.
"""
import sys

for _p in ("/opt/trn_rl_repo", "/root/.axon_site", "/root/.axon_site/_ro/trn_rl_repo",
           "/root/.axon_site/_ro/pypackages"):
    if _p not in sys.path:
        sys.path.append(_p)

from contextlib import ExitStack

import ml_dtypes
import numpy as np

import concourse.bass as bass  # noqa: F401
import concourse.tile as tile
from concourse import bacc, mybir
from concourse import bass_utils
from concourse.bass_interp import get_hw_module
from concourse.masks import make_identity

N_CORES = 8
B, N, C = 16, 2049, 1024
B_LOC = B // N_CORES        # 2 examples per core
P = 128                     # SBUF partitions
CT = C // P                 # 8 c tiles
NT = 16                     # logits per partition (128*16 = 2048 rows)
G = 4                       # rows per x DMA group -> [128, 4096] tiles
NG = NT // G
F32 = mybir.dt.float32
BF16 = mybir.dt.bfloat16
NP_BF16 = ml_dtypes.bfloat16


def build_nc():
    nc = bacc.Bacc("TRN2", target_bir_lowering=False, debug=False,
                   enable_asserts=True, num_devices=N_CORES)

    x_d = nc.dram_tensor("x", [B_LOC, N, C], BF16, kind="ExternalInput").ap()
    wq_d = nc.dram_tensor("wq", [C, C], BF16, kind="ExternalInput").ap()
    wkt_d = nc.dram_tensor("wkt", [C, C], BF16, kind="ExternalInput").ap()
    o_d = nc.dram_tensor("o", [B_LOC, N - 1], F32, kind="ExternalOutput").ap()

    with tile.TileContext(nc) as tc, ExitStack() as ctx:
        sing = ctx.enter_context(tc.tile_pool(name="sing", bufs=1))
        xp = ctx.enter_context(tc.tile_pool(name="xp", bufs=8))
        scr = ctx.enter_context(tc.tile_pool(name="scr", bufs=2))
        pst = ctx.enter_context(tc.tile_pool(name="pst", bufs=2, space="PSUM"))
        pss = ctx.enter_context(tc.tile_pool(name="pss", bufs=2, space="PSUM"))
        dram = ctx.enter_context(tc.tile_pool(name="dram", bufs=1, space="DRAM"))

        # --- x0^T [128, 2*8] gather heads the SWDGE queue ------------------
        x0T = sing.tile([P, B_LOC * CT], BF16, tag="x0T")
        for b in range(B_LOC):
            nc.gpsimd.dma_start(
                x0T[:].rearrange("p (j b) -> p b j", b=B_LOC)[:, b, :],
                x_d[b, 0, :].rearrange("(j p) -> p j", p=P))

        # --- weights on the ACT HW queue (land early, off the x stream) ----
        # wq_sb columns [1024j : 1024(j+1)] hold W_q rows-tile j (c_in chunk).
        # wkt_sb columns [1024m : 1024(m+1)] hold W_k^T rows-tile m (c_out).
        wq_sb = sing.tile([P, CT * C], BF16, tag="wq_sb")
        wkt_sb = sing.tile([P, CT * C], BF16, tag="wkt_sb")
        for j in range(CT):
            nc.scalar.dma_start(wq_sb[:, C * j:C * (j + 1)],
                                wq_d[P * j:P * (j + 1), :])
        for j in range(CT):
            nc.scalar.dma_start(wkt_sb[:, C * j:C * (j + 1)],
                                wkt_d[P * j:P * (j + 1), :])

        ident = sing.tile([P, P], F32, tag="ident")
        make_identity(nc, ident[:])
        ones128 = sing.tile([P, P], F32, tag="ones128")
        nc.gpsimd.memset(ones128[:], 1.0)
        ones1 = sing.tile([1, P], BF16, tag="ones1")
        nc.gpsimd.memset(ones1[:], 1.0)

        # --- x stream split across BOTH HW queues (weights ride SWDGE) -----
        xts = []
        for b in range(B_LOC):
            xb = x_d[b, 1:N, :].rearrange("(p f) c -> p (f c)", f=NT)
            for g in range(NG):
                xt = xp.tile([P, G * C], BF16, tag="xg", name=f"xg{b}_{g}")
                nc.sync.dma_start(xt[:], xb[:, G * C * g:G * C * (g + 1)])
                xts.append(xt)

        # --- q_cls [2, 1024]: rhs-streaming bf16 matmuls -------------------
        q_sb = sing.tile([B_LOC, C], F32, tag="q_sb")
        for h in range(2):
            psq = pss.tile([B_LOC, 512], F32, tag="psq")
            for j in range(CT):
                nc.tensor.matmul(psq[:],
                                 x0T[:, B_LOC * j:B_LOC * (j + 1)],
                                 wq_sb[:, C * j + 512 * h:C * j + 512 * (h + 1)],
                                 start=(j == 0), stop=(j == CT - 1))
            nc.scalar.copy(q_sb[:, 512 * h:512 * (h + 1)], psq[:])

        # --- q_cls^T [128, 2*8] via PE transposes, cast to bf16 ------------
        qT = sing.tile([P, B_LOC * CT], BF16, tag="qT")
        for m in range(CT):
            ps = pst.tile([P, B_LOC], F32, tag="tp")
            nc.tensor.transpose(ps[:], q_sb[:, P * m:P * (m + 1)],
                                ident[:B_LOC, :B_LOC])
            nc.scalar.copy(qT[:, B_LOC * m:B_LOC * (m + 1)], ps[:])

        # --- t rows per example [1, 1024] (M=1 so base partition stays 0) --
        t_row = [sing.tile([1, C], BF16, tag=f"t_row{b}", name=f"t_row{b}")
                 for b in range(B_LOC)]
        for b in range(B_LOC):
            for h in range(2):
                ps_t = pss.tile([1, 512], F32, tag="psq")
                for m in range(CT):
                    nc.tensor.matmul(ps_t[:],
                                     qT[:, B_LOC * m + b:B_LOC * m + b + 1],
                                     wkt_sb[:, C * m + 512 * h:C * m + 512 * (h + 1)],
                                     start=(m == 0), stop=(m == CT - 1))
                nc.scalar.copy(t_row[b][:, 512 * h:512 * (h + 1)], ps_t[:])

        # --- broadcast t across partitions via K=1 all-ones matmuls --------
        tb = []
        for b in range(B_LOC):
            tb_b = sing.tile([P, C], BF16, tag=f"tb{b}", name=f"tb{b}")
            for h in range(2):
                ps_b = pss.tile([P, 512], F32, tag="pstb")
                nc.tensor.matmul(ps_b[:], ones1[:],
                                 t_row[b][:, 512 * h:512 * (h + 1)],
                                 start=True, stop=True)
                nc.scalar.copy(tb_b[:, 512 * h:512 * (h + 1)], ps_b[:])
            tb.append(tb_b)

        # --- big pass: fused row-dot products + softmax per example --------
        for b in range(B_LOC):
            Lb = sing.tile([P, NT], F32, tag=f"L{b}", name=f"L{b}")
            for g in range(NG):
                xt = xts[b * NG + g]
                for l in range(G):
                    f = G * g + l
                    s = scr.tile([P, C], BF16, tag="scr")
                    nc.vector.scalar_tensor_tensor(
                        out=s[:], in0=xt[:, C * l:C * (l + 1)], scalar=1.0,
                        in1=tb[b][:],
                        op0=mybir.AluOpType.mult, op1=mybir.AluOpType.mult,
                        accum_out=Lb[:, f:f + 1])

            E = sing.tile([P, NT], F32, tag=f"E{b}", name=f"E{b}")
            S = sing.tile([P, 1], F32, tag=f"S{b}", name=f"S{b}")
            nc.scalar.activation(E[:], Lb[:], mybir.ActivationFunctionType.Exp,
                                 bias=0.0, scale=float(C ** -0.5), accum_out=S[:])
            psS = pss.tile([P, 1], F32, tag="psS")
            nc.tensor.matmul(psS[:], ones128[:], S[:], start=True, stop=True)
            Rv = sing.tile([P, 1], F32, tag=f"R{b}", name=f"R{b}")
            nc.vector.reciprocal(Rv[:], psS[:])
            Pb = sing.tile([P, NT], F32, tag=f"P{b}", name=f"P{b}")
            nc.vector.tensor_scalar_mul(Pb[:], E[:], Rv[:])
            nc.scalar.dma_start(o_d[b].rearrange("(p f) -> p f", f=NT), Pb[:])

    nc.compile()
    nc.m = get_hw_module(nc.m)
    return nc


_NC_CACHE = {}


def _get_nc():
    if "nc" not in _NC_CACHE:
        _NC_CACHE["nc"] = build_nc()
    return _NC_CACHE["nc"]


def _prep_inputs(x, w_qkv):
    """Host-side shard/layout prep: bf16 cast; W_q natural, W_k transposed."""
    x_bf = np.asarray(x, dtype=np.float32).astype(NP_BF16)
    w = np.asarray(w_qkv, dtype=np.float32)
    wq = np.ascontiguousarray(w[:, :C]).astype(NP_BF16)
    wkt = np.ascontiguousarray(w[:, C:2 * C].T).astype(NP_BF16)
    return x_bf, wq, wkt


def _run(x, w_qkv, **kwargs):
    assert np.asarray(x).shape == (B, N, C)
    x_bf, wq, wkt = _prep_inputs(x, w_qkv)
    nc = _get_nc()
    in_maps = [{"x": x_bf[c * B_LOC:(c + 1) * B_LOC], "wq": wq, "wkt": wkt}
               for c in range(N_CORES)]
    res = bass_utils.run_bass_kernel_spmd(nc, in_maps,
                                          core_ids=list(range(N_CORES)), **kwargs)
    out = np.concatenate([res.results[c]["o"] for c in range(N_CORES)], axis=0)
    return out, res


def kernel(x, w_qkv):
    out, _ = _run(x, w_qkv)
    return out



# revision 7
# speedup vs baseline: 1.5558x; 1.5558x over previous
"""Trainium2 Bass kernel for nn_Attention_59691455480358 (sparse CLS attention).

Math: the reference computes softmax over
    logits[b, n] = (x[b,0]@W_q) . (x[b,1+n]@W_k) * C^-0.5,  n in [0, 2048).
Only the CLS query row matters and V is unused, so fold the K-projection into
the query side:

    q[b]        = x[b,0,:] @ W_q                  # [C]
    t[b]        = W_k_storage @ q[b]              # [C]  (t[c] = sum_m Wk[c,m] q[m])
    logits[b,n] = x[b,1+n,:] . t[b]
    out[b]      = softmax(logits[b] * C^-0.5)

Sharding: pure data parallel - batch 16 over 8 NeuronCores (2 examples/core).

Device mapping (v2 - PE-centric, DMA-overlapped):
  * x ships host-transposed per core as xt[j, p, b*2048+n] = x[b, 1+n, 128j+p]
    (bf16), so channels sit on SBUF partitions.  The row-dot pass then runs on
    the Tensor engine: logits accumulate over the 8 channel chunks j as
    matmul(psum[1,512], lhsT=tT[:,2j+b], rhs=xt_j[:, b*2048+512s:...]), fully
    overlapped with the streaming x DMA (PE consumes each 1 MB chunk as it
    lands).
  * All input DMAs ride one HWDGE FIFO queue in priority order: x0^T (4 KB),
    W_q (2 MB), W_k^T (2 MB), then 8x 1MB xt chunks.  Weights land first so
    the small q->t chain (16+16 M=2 matmuls + 16 PE transposes) finishes
    while x still streams.
  * Softmax per example on the single logit partition: slice-wise ACT exp with
    fused partial sums, DVE reciprocal, then the normalize multiply split
    DVE(b0)/ACT(b1); outputs go out on the two HWDGE queues in parallel.
No max-subtraction in softmax: scaled logits are ~N(0,1) (weights are
1/sqrt(C)-scaled gaussians), exp cannot overflow fp32.
"""
import sys

for _p in ("/opt/trn_rl_repo", "/root/.axon_site", "/root/.axon_site/_ro/trn_rl_repo",
           "/root/.axon_site/_ro/pypackages"):
    if _p not in sys.path:
        sys.path.append(_p)

from contextlib import ExitStack

import ml_dtypes
import numpy as np

import concourse.bass as bass  # noqa: F401
import concourse.tile as tile
from concourse import bacc, mybir
from concourse import bass_utils
from concourse.bass_interp import get_hw_module
from concourse.masks import make_identity

N_CORES = 8
B, N, C = 16, 2049, 1024
B_LOC = B // N_CORES        # 2 examples per core
P = 128                     # SBUF partitions
CT = C // P                 # 8 channel chunks
NR = N - 1                  # 2048 key rows per example
SL = 512                    # logit slice (one PSUM bank of fp32)
NS = NR // SL               # 4 slices per example
F32 = mybir.dt.float32
BF16 = mybir.dt.bfloat16
NP_BF16 = ml_dtypes.bfloat16


def build_nc():
    nc = bacc.Bacc("TRN2", target_bir_lowering=False, debug=False,
                   enable_asserts=True, num_devices=N_CORES)

    x0t_d = nc.dram_tensor("x0t", [P, CT * B_LOC], BF16, kind="ExternalInput").ap()
    wq_d = nc.dram_tensor("wq", [P, CT * C], BF16, kind="ExternalInput").ap()
    wkt_d = nc.dram_tensor("wkt", [P, CT * C], BF16, kind="ExternalInput").ap()
    xt_d = nc.dram_tensor("xt", [CT, P, B_LOC * NR], BF16, kind="ExternalInput").ap()
    o_d = nc.dram_tensor("o", [B_LOC, NR], F32, kind="ExternalOutput").ap()

    with tile.TileContext(nc) as tc, ExitStack() as ctx:
        sing = ctx.enter_context(tc.tile_pool(name="sing", bufs=1))
        xp = ctx.enter_context(tc.tile_pool(name="xp", bufs=CT))

        # ---- input DMAs, one FIFO queue, priority order --------------------
        x0t = sing.tile([P, CT * B_LOC], BF16, tag="x0t")
        nc.sync.dma_start(x0t[:], x0t_d)
        wq_sb = sing.tile([P, CT * C], BF16, tag="wq_sb")
        nc.sync.dma_start(wq_sb[:], wq_d)
        wkt_sb = sing.tile([P, CT * C], BF16, tag="wkt_sb")
        nc.sync.dma_start(wkt_sb[:], wkt_d)
        xts = []
        for j in range(CT):
            xt = xp.tile([P, B_LOC * NR], BF16, tag="xg", name=f"xt{j}")
            nc.sync.dma_start(xt[:], xt_d[j])
            xts.append(xt)

        ident = sing.tile([P, P], F32, tag="ident")
        make_identity(nc, ident[:])

        qT = sing.tile([P, B_LOC * CT], BF16, tag="qT")
        tT = sing.tile([P, B_LOC * CT], BF16, tag="tT")
        with tc.tile_pool(name="pse", bufs=2, space="PSUM") as pse:
            # ---- q = x0 @ W_q  -> [2, 1024] fp32 ---------------------------
            q_sb = sing.tile([B_LOC, C], F32, tag="q_sb")
            for h in range(2):
                psq = pse.tile([B_LOC, SL], F32, tag="psq")
                for j in range(CT):
                    nc.tensor.matmul(psq[:], x0t[:, B_LOC * j:B_LOC * (j + 1)],
                                     wq_sb[:, C * j + SL * h:C * j + SL * (h + 1)],
                                     start=(j == 0), stop=(j == CT - 1))
                nc.scalar.copy(q_sb[:, SL * h:SL * (h + 1)], psq[:])

            # ---- q^T [128, 2*8] bf16 via PE transposes ---------------------
            for m in range(CT):
                pst = pse.tile([P, B_LOC], F32, tag="pst")
                nc.tensor.transpose(pst[:], q_sb[:, P * m:P * (m + 1)],
                                    ident[:B_LOC, :B_LOC])
                nc.scalar.copy(qT[:, B_LOC * m:B_LOC * (m + 1)], pst[:])

            # ---- t = q @ Wk_storage^T -> [2, 1024] fp32 --------------------
            t_sb = sing.tile([B_LOC, C], F32, tag="t_sb")
            for h in range(2):
                pst2 = pse.tile([B_LOC, SL], F32, tag="psq")
                for m in range(CT):
                    nc.tensor.matmul(pst2[:], qT[:, B_LOC * m:B_LOC * (m + 1)],
                                     wkt_sb[:, C * m + SL * h:C * m + SL * (h + 1)],
                                     start=(m == 0), stop=(m == CT - 1))
                nc.scalar.copy(t_sb[:, SL * h:SL * (h + 1)], pst2[:])

            # ---- t^T [128, 2*8] bf16 via PE transposes ---------------------
            for j in range(CT):
                pstt = pse.tile([P, B_LOC], F32, tag="pst")
                nc.tensor.transpose(pstt[:], t_sb[:, P * j:P * (j + 1)],
                                    ident[:B_LOC, :B_LOC])
                nc.scalar.copy(tT[:, B_LOC * j:B_LOC * (j + 1)], pstt[:])

        # ---- row-dot pass on PE: 8 interleaved accumulation groups ---------
        ps = ctx.enter_context(tc.tile_pool(name="psl", bufs=8, space="PSUM"))
        psL = [[ps.tile([1, SL], F32, tag="psL", name=f"L{b}_{s}")
                for s in range(NS)] for b in range(B_LOC)]
        for j in range(CT):
            for b in range(B_LOC):
                for s in range(NS):
                    nc.tensor.matmul(
                        psL[b][s][:],
                        tT[:, B_LOC * j + b:B_LOC * j + b + 1],
                        xts[j][:, NR * b + SL * s:NR * b + SL * (s + 1)],
                        start=(j == 0), stop=(j == CT - 1))

        # ---- softmax per example on partition 0 ----------------------------
        scale = float(C ** -0.5)
        for b in range(B_LOC):
            E = sing.tile([1, NR], F32, tag=f"E{b}", name=f"E{b}")
            S4 = sing.tile([1, NS], F32, tag=f"S4_{b}", name=f"S4_{b}")
            for s in range(NS):
                nc.scalar.activation(E[:, SL * s:SL * (s + 1)], psL[b][s][:],
                                     mybir.ActivationFunctionType.Exp,
                                     bias=0.0, scale=scale,
                                     accum_out=S4[:, s:s + 1])
            S4c = sing.tile([1, NS], F32, tag=f"S4c{b}", name=f"S4c{b}")
            Ssum = sing.tile([1, 1], F32, tag=f"Ss{b}", name=f"Ss{b}")
            nc.scalar.activation(S4c[:], S4[:],
                                 mybir.ActivationFunctionType.Copy,
                                 accum_out=Ssum[:])
            R = sing.tile([1, 1], F32, tag=f"R{b}", name=f"R{b}")
            nc.vector.reciprocal(R[:], Ssum[:])
            Pb = sing.tile([1, NR], F32, tag=f"P{b}", name=f"P{b}")
            if b == 0:
                nc.vector.tensor_scalar_mul(Pb[:], E[:], R[:])
                nc.sync.dma_start(o_d[b], Pb[:])
            else:
                nc.scalar.activation(Pb[:], E[:],
                                     mybir.ActivationFunctionType.Copy,
                                     bias=0.0, scale=R[:])
                nc.scalar.dma_start(o_d[b], Pb[:])

    nc.compile()
    nc.m = get_hw_module(nc.m)
    return nc


_NC_CACHE = {}


def _get_nc():
    if "nc" not in _NC_CACHE:
        _NC_CACHE["nc"] = build_nc()
    return _NC_CACHE["nc"]


def _prep_inputs(x, w_qkv):
    """Host-side shard/layout prep: bf16 cast + channel-major transpose."""
    x_bf = np.asarray(x, dtype=np.float32).astype(NP_BF16)
    w = np.asarray(w_qkv, dtype=np.float32)
    # chunk-interleaved SBUF layout: w_sb[p, j*C + m] = W[128j + p, m]
    wq = np.ascontiguousarray(
        w[:, :C].reshape(CT, P, C).transpose(1, 0, 2).reshape(P, CT * C)
    ).astype(NP_BF16)
    wkt = np.ascontiguousarray(
        w[:, C:2 * C].T.reshape(CT, P, C).transpose(1, 0, 2).reshape(P, CT * C)
    ).astype(NP_BF16)
    # [C, B, NR] channel-major view of the key rows
    xt_all = np.ascontiguousarray(x_bf[:, 1:, :].transpose(2, 0, 1))
    x0_all = x_bf[:, 0, :]  # [B, C]
    return x_bf, wq, wkt, xt_all, x0_all


def _run(x, w_qkv, **kwargs):
    assert np.asarray(x).shape == (B, N, C)
    _, wq, wkt, xt_all, x0_all = _prep_inputs(x, w_qkv)
    nc = _get_nc()
    in_maps = []
    for c in range(N_CORES):
        sl = slice(c * B_LOC, (c + 1) * B_LOC)
        xt = np.ascontiguousarray(xt_all[:, sl, :]).reshape(CT, P, B_LOC * NR)
        x0t = np.ascontiguousarray(
            x0_all[sl].T.reshape(CT, P, B_LOC).transpose(1, 0, 2)
        ).reshape(P, CT * B_LOC)
        in_maps.append({"x0t": x0t, "wq": wq, "wkt": wkt, "xt": xt})
    res = bass_utils.run_bass_kernel_spmd(nc, in_maps,
                                          core_ids=list(range(N_CORES)), **kwargs)
    out = np.concatenate([res.results[c]["o"] for c in range(N_CORES)], axis=0)
    return out, res


def kernel(x, w_qkv):
    out, _ = _run(x, w_qkv)
    return out
